# revision 23
# baseline (speedup 1.0000x reference)
"""Trainium2 Bass kernel for nn_BlockCorrelation (sparse/block attention).

Self-contained: accepts FULL inputs, shards across 8 NeuronCores internally,
returns the FULL output.

Math (see reference):
    feat = x.mean((2,3)); feat_n = LN(feat)*ln_w + ln_b
    qkv  = feat_n @ in_proj_w.T + in_proj_b  -> 8-head attention with a
    block-diagonal mask (attend only within equal batch_indices groups)
    out  = attn_out @ out_proj_w.T + out_proj_b
    y    = x + where(group_count>1, gamma*out, 0)[..., None, None]

Distribution / key optimizations:
  - rows sorted by group on host; core c owns sorted tiles c and c+8
    (interleaved, so each half of the sorted rows is one AllGather).
  - x is shipped bf16 and transposed to [rows, HW, C]: with C innermost both
    the pooling tree (packed pairwise adds over HW) and the y = x + g*delta
    broadcast-add run in the DVE 2x mode; a [128,8,1024] chunk is one 16 KB
    contiguous run per partition.  y returns bf16 (host upcasts/permutes).
  - x rides the scalar-engine DMA queue alone (weights/masks on the gpsimd
    queue) so the stream saturates from t=0; the add-phase prefetch reuses
    the same SBUF ring and is gated behind AG(B) so the critical-path
    AllGather never shares HBM bandwidth with the prefetch stream.
  - a tiny warmup AllGather at t=0 bootstraps the CC channel.
  - attention is head-parallel (core h = head h over all rows), entirely in
    fp8 (e4m3) on the PE: LN'd featT is AllGathered in fp8 (1 MiB), QKV /
    banded scores + one-hot mask / exp / ones-matmul denominator all fp8;
    out[q,d] = expT @ v is divided by the denominator on the head core and
    routed to row owners by two AllToAlls (256 KB bf16 each, one per row
    tile) so out_proj/gamma/adds for tile 0 start while tile 1's attention
    is still finishing.  Attention feeds only gamma*delta (~0.5% of |y|),
    so fp8 error is negligible.
  - 1-element weight "touches" and zero-bias pins force the static Tile
    scheduler to keep the half-B LN/AllGather chain ahead of the
    AllGather-gated attention ops in each in-order engine stream.
"""

import json
import sys

if "/opt/trn_rl_repo" not in sys.path:
    sys.path.insert(0, "/opt/trn_rl_repo")

import ml_dtypes
import numpy as np

import concourse.bass as bass
import concourse.mybir as mybir
import concourse.tile as tile
from concourse.bass_utils import run_bass_kernel_spmd
from concourse.masks import make_identity

F32 = mybir.dt.float32
BF16 = mybir.dt.bfloat16
F8 = mybir.dt.float8e4

# Problem shape (hardcoded per contract)
N, C, HW = 2048, 1024, 64
NH, HD = 8, 128
NG = 32
EPS = 1e-5
NCORES = 8
NS = N // NCORES          # 256 rows per core
NT = 2                    # own row tiles of 128 (tile k = global tile c+8k)
CB = C // 128             # 8 channel blocks
MT = N // 128             # 16 global key tiles
HMT = MT // 2             # 8 key tiles per half
NQC = N // 512            # 4 query chunks of 512
MASK_NEG = -50.0          # additive mask magnitude
GCAP = 128                # band attention assumes group fits in one m-tile
WC = 8                    # hw-chunks per row tile (64 hw / 8)
PF = 7                    # add-phase x prefetch depth (chunks of 2 MiB)


def _band(j, dense):
    if dense:
        return range(MT)
    return range(max(0, 4 * j - 1), min(MT, 4 * j + 5))


# ---------------------------------------------------------------------------
# walrus workaround: this build rejects >1 sem wait per instruction in some
# CTRL lowerings; split excess on_wait entries onto preceding same-engine
# EventSemaphore instructions (the exact shape wait_ge() lowers to).
def _split_waits_json(j, max_waits=1):
    for f in j.get("functions", []):
        for bb in f.get("blocks", []):
            out = []
            for ins in bb.get("instructions", []):
                si = ins.get("sync_info")
                waits = (si or {}).get("on_wait") or []
                if len(waits) > max_waits:
                    head, tail = waits[:-max_waits], waits[-max_waits:]
                    for k, w in enumerate(head):
                        out.append({
                            "name": f"{ins['name']}-wsplit{k}",
                            "opcode": "EventSemaphore",
                            "engine": ins["engine"],
                            "ins": [],
                            "outs": [],
                            "debug": ins.get("debug", 0),
                            "sync_info": {"on_update": [], "on_wait": [w]},
                        })
                    si["on_wait"] = tail
                out.append(ins)
            bb["instructions"] = out
    return j


def _install_wait_split(nc, max_waits=1):
    def to_json_bytes_fixed():
        j = json.loads(mybir.module_to_json_bytes(nc.m))
        return json.dumps(_split_waits_json(j, max_waits)).encode()

    nc.to_json_bytes = to_json_bytes_fixed


def _bcast_ap(ap, parts=128):
    """DRAM AP broadcast across partitions (stride-0 partition dim)."""
    return bass.AP(tensor=ap.tensor, offset=ap.offset, ap=[[0, parts]] + ap.ap)


# ---------------------------------------------------------------------------
def build_program(dense=False):
    nc = bass.Bass(num_devices=NCORES)

    # --- per-core parameters (SPMD: same program, different data) ---
    xs = nc.declare_dram_parameter("xs", [NS, HW, C], BF16, isOutput=False)
    wq = nc.declare_dram_parameter("wq", [C, HD], F8, isOutput=False)
    wk = nc.declare_dram_parameter("wk", [C, HD], F8, isOutput=False)
    wv = nc.declare_dram_parameter("wv", [C, HD], F8, isOutput=False)
    qb = nc.declare_dram_parameter("qb", [HD, 1], F32, isOutput=False)  # pre-scaled by 1/sqrt(HD)
    kb = nc.declare_dram_parameter("kb", [HD, 1], F32, isOutput=False)
    vb = nc.declare_dram_parameter("vb", [1, HD], BF16, isOutput=False)
    wot = nc.declare_dram_parameter("wot", [C, C], F8, isOutput=False)  # out_proj_w.T (ci, co)
    oha = nc.declare_dram_parameter("oha", [NG, N], F8, isOutput=False)
    ohb = nc.declare_dram_parameter("ohb", [NG, N], F8, isOutput=False)
    bo = nc.declare_dram_parameter("bo", [C], F32, isOutput=False)
    gsh = nc.declare_dram_parameter("gsh", [NS], F32, isOutput=False)
    out = nc.declare_dram_parameter("out", [NS, HW, C], BF16, isOutput=True)

    # --- internal DRAM for collectives ---
    warm_in = nc.dram_tensor("warm_in", [16], BF16)
    warm_out = nc.dram_tensor("warm_out", [NCORES * 16], BF16,
                              addr_space="Shared")
    featT_sh = [nc.dram_tensor(f"featT_sh{h}", [C, 128], F8)
                for h in range(NT)]
    featT_full = [
        nc.dram_tensor(f"featT_full{h}", [NCORES * C, 128], F8,
                       addr_space="Shared")
        for h in range(NT)
    ]
    a2a_in = [nc.dram_tensor(f"a2a_in{k}", [N // 2, HD], BF16)
              for k in range(NT)]
    a2a_out = [nc.dram_tensor(f"a2a_out{k}", [N // 2, HD], BF16)
               for k in range(NT)]

    groups = [list(range(NCORES))]
    inv_sqrt_hd = 1.0 / float(np.sqrt(np.float32(HD)))
    BW = MT if dense else 6
    pf = 4 if dense else PF

    with tile.TileContext(nc, num_cores=NCORES) as tc:
      with (
        tc.tile_pool(name="singles", bufs=1) as singles,
        tc.tile_pool(name="attper", bufs=1) as attper,
        tc.tile_pool(name="qkvw", bufs=1) as qkvw,
        tc.tile_pool(name="maskp", bufs=1) as maskp,
        tc.tile_pool(name="expbuf", bufs=2) as expbuf,
        tc.tile_pool(name="p3", bufs=1) as p3,
        tc.tile_pool(name="ftp", bufs=2) as ftp,
        tc.tile_pool(name="qkvps", bufs=2, space="PSUM") as qkvps,
        tc.tile_pool(name="scoreps", bufs=2, space="PSUM") as scoreps,
        tc.tile_pool(name="avdps", bufs=2, space="PSUM") as avdps,
        tc.tile_pool(name="denps", bufs=1, space="PSUM") as denps,
      ):
        # ---- CC-channel warmup: bootstrap the collectives rendezvous now so
        # the first real AllGather doesn't pay it.
        nc.gpsimd.collective_compute(
            "AllGather", mybir.AluOpType.bypass, replica_groups=groups,
            ins=[warm_in[:]], outs=[warm_out[:]])

        # ---- constants / weights (weights on the gpsimd DMA queue; the
        # scalar+tensor queues carry only the x stream) ----
        ident = singles.tile([128, 128], F32)
        make_identity(nc, ident)
        identb = singles.tile([128, 128], BF16)
        make_identity(nc, identb)
        ones_col = singles.tile([128, 1], F8)
        nc.vector.memset(ones_col, 1.0)
        one_1x1 = singles.tile([1, 1], F32)
        nc.vector.memset(one_1x1, 1.0)
        ones_row = singles.tile([1, 128], BF16)
        nc.vector.memset(ones_row, 1.0)

        wq_t = qkvw.tile([128, CB, HD], F8)
        nc.gpsimd.dma_start(out=wq_t, in_=wq.rearrange("(cb p) d -> p cb d", p=128))
        wk_t = qkvw.tile([128, CB, HD], F8)
        nc.gpsimd.dma_start(out=wk_t, in_=wk.rearrange("(cb p) d -> p cb d", p=128))
        wv_t = qkvw.tile([128, CB, HD], F8)
        nc.gpsimd.dma_start(out=wv_t, in_=wv.rearrange("(cb p) d -> p cb d", p=128))
        qb_t = qkvw.tile([128, 1], F32)
        nc.gpsimd.dma_start(out=qb_t, in_=qb[:])
        kb_t = qkvw.tile([128, 1], F32)
        nc.gpsimd.dma_start(out=kb_t, in_=kb[:])
        vb_t = qkvw.tile([1, 128], BF16)
        nc.gpsimd.dma_start(out=vb_t, in_=vb[:])
        wo_t = qkvw.tile([128, NH, C], F8)   # (ci within head-block, h, co)
        nc.gpsimd.dma_start(out=wo_t, in_=wot.rearrange("(h p) co -> p h co", p=128))
        oha_t = maskp.tile([128, N], F8)
        nc.vector.memset(oha_t, 0.0)
        nc.gpsimd.dma_start(out=oha_t[:NG, :], in_=oha[:])
        ohb_t = maskp.tile([128, N], F8)
        nc.vector.memset(ohb_t, 0.0)
        nc.gpsimd.dma_start(out=ohb_t[:NG, :], in_=ohb[:])
        bo_t = p3.tile([128, C], F32)
        nc.gpsimd.dma_start(out=bo_t, in_=_bcast_ap(bo[:]))
        g_t = p3.tile([128, NT], F32)
        nc.gpsimd.dma_start(out=g_t, in_=gsh.rearrange("(t p) -> p t", p=128))

        # ---- persistent attention state ----
        qT = attper.tile([128, N], F8, name="qT")
        kT = attper.tile([128, N], F8, name="kT")
        v_t = attper.tile([128, MT, HD], F8, name="v")
        outb = attper.tile([128, MT, HD], BF16, name="outb")
        denT = attper.tile([128, MT], F32, name="denT")
        recipT = attper.tile([128, MT], F32, name="recipT")
        den_sb = attper.tile([1, N], F32, name="den_sb")
        out_all = attper.tile([128, NT, C], BF16, name="out_all")
        out_allT = attper.tile([128, NT, NH, 128], F8, name="out_allT")
        gd = p3.tile([128, NT, C], BF16, name="gd")
        # scheduler pins: qb2/kb2/vz depend on half-B's LN output, so the
        # static per-engine schedule cannot hoist attention-phase DVE/Act
        # ops above the half-B pooling/LN stream (a hoisted wait stalls the
        # in-order engine and delays AG(B) by ~30us).
        qb2 = attper.tile([128, 1], F32, name="qb2")
        kb2 = attper.tile([128, 1], F32, name="kb2")
        vz = attper.tile([128, 1], F32, name="vz")

        # ---- phase-1 pools (exit before the prefetch pool opens) ----
        xin_cm = tc.tile_pool(name="xin", bufs=pf)
        p1_cm = tc.tile_pool(name="p1", bufs=2)
        p1one_cm = tc.tile_pool(name="p1one", bufs=1)
        xin = xin_cm.__enter__()
        p1 = p1_cm.__enter__()
        p1one = p1one_cm.__enter__()

        eps_t = p1one.tile([128, 1], F32)
        nc.vector.memset(eps_t, EPS * HW * HW)  # LN on sums: eps * 64^2

        faccs = {}

        def xq(i):
            return nc.scalar

        def xq2(i):
            return nc.sync

        def pool_loads_reduces(k):
            """Stream own tile k as [128, 8hw, 1024c] chunks; 2x-packed
            pairwise-add tree over hw; accumulate channel sums in bf16."""
            facc = p1.tile([128, C], BF16, tag="facc", name="facc")
            facc2 = p1.tile([128, C], BF16, tag="facc2", name="facc2")
            faccs[k] = (facc, facc2)

            def finish(xt, wc):
                acc = facc if wc % 2 == 0 else facc2
                nc.vector.tensor_tensor(
                    out=xt[:, 0:2, :], in0=xt[:, 0:2, :],
                    in1=xt[:, 2:4, :], op=mybir.AluOpType.add)
                if wc < 2:
                    nc.vector.tensor_tensor(
                        out=acc, in0=xt[:, 0, :], in1=xt[:, 1, :],
                        op=mybir.AluOpType.add)
                else:
                    t3 = p1.tile([128, C], BF16, tag="t3", name="t3")
                    nc.vector.tensor_tensor(
                        out=t3, in0=xt[:, 0, :], in1=xt[:, 1, :],
                        op=mybir.AluOpType.add)
                    nc.vector.tensor_tensor(
                        out=acc, in0=acc, in1=t3,
                        op=mybir.AluOpType.add)

            pend = []
            with nc.allow_low_precision(reason="pool partials in bf16"):
                for wc in range(WC):
                    xt = xin.tile([128, WC, C], BF16, tag="xt", name="xt")
                    xq(wc).dma_start(
                        out=xt,
                        in_=xs[k * 128:(k + 1) * 128,
                               wc * WC:(wc + 1) * WC, :])
                    nc.vector.tensor_tensor(
                        out=xt[:, 0:4, :], in0=xt[:, 0:4, :],
                        in1=xt[:, 4:8, :], op=mybir.AluOpType.add)
                    pend.append((xt, wc))
                    if len(pend) > 2:
                        finish(*pend.pop(0))
                for p_ in pend:
                    finish(*p_)

        def ln_transpose_ag(k):
            """LN over channels (on sums), transpose, AG half k (fp8)."""
            facc, facc2 = faccs[k]
            with nc.allow_low_precision(reason="pool partials in bf16"):
                nc.vector.tensor_tensor(
                    out=facc, in0=facc, in1=facc2, op=mybir.AluOpType.add)
            stats = p1.tile([128, 2, 6], F32, tag="stats", name="stats")
            for sg in range(2):
                nc.vector.bn_stats(out=stats[:, sg, :],
                                   in_=facc[:, sg * 512:(sg + 1) * 512])
            mv = p1.tile([128, 2], F32, tag="mv", name="mv")
            nc.vector.bn_aggr(out=mv, in_=stats)
            std = p1.tile([128, 1], F32, tag="std", name="std")
            nc.scalar.activation(
                out=std, in_=mv[:, 1:2],
                func=mybir.ActivationFunctionType.Sqrt, bias=eps_t, scale=1.0)
            rstd = p1.tile([128, 1], F32, tag="rstd", name="rstd")
            nc.vector.reciprocal(out=rstd, in_=std)
            featn = p1.tile([128, C], F32, tag="featn", name="featn")
            nc.vector.tensor_scalar(
                out=featn, in0=facc, scalar1=mv[:, 0:1], scalar2=rstd,
                op0=mybir.AluOpType.subtract, op1=mybir.AluOpType.mult)
            fTb = p1.tile([128, CB, 128], F8, tag="fTb", name="fTb")
            for cbi in range(CB):
                pt = scoreps.tile([128, 128], F32, tag="pscore", name="trps")
                nc.tensor.transpose(pt, featn[:, cbi * 128:(cbi + 1) * 128], ident)
                nc.vector.tensor_copy(out=fTb[:, cbi, :], in_=pt)
            nc.gpsimd.dma_start(
                out=featT_sh[k].rearrange("(cb p) n -> p cb n", p=128),
                in_=fTb)
            nc.gpsimd.collective_compute(
                "AllGather", mybir.AluOpType.bypass, replica_groups=groups,
                ins=[featT_sh[k][:]], outs=[featT_full[k][:]])
            if k == NT - 1:
                nc.vector.scalar_tensor_tensor(
                    out=vz, in0=fTb[:, 0, 0:1], scalar=0.0, in1=fTb[:, 0, 0:1],
                    op0=mybir.AluOpType.mult, op1=mybir.AluOpType.mult)
                nc.vector.scalar_tensor_tensor(
                    out=qb2, in0=fTb[:, 0, 0:1], scalar=0.0, in1=qb_t,
                    op0=mybir.AluOpType.mult, op1=mybir.AluOpType.add)
                nc.vector.scalar_tensor_tensor(
                    out=kb2, in0=fTb[:, 0, 0:1], scalar=0.0, in1=kb_t,
                    op0=mybir.AluOpType.mult, op1=mybir.AluOpType.add)
                # 1-element touches: every attention matmul reads one of
                # these weights, so the tensor stream cannot be scheduled
                # ahead of half-B's LN/transposes (which gate AG(B)).
                for wt in (wq_t, wk_t, wv_t):
                    nc.vector.tensor_scalar(
                        out=wt[0:1, 0, 0:1], in0=wt[0:1, 0, 0:1],
                        scalar1=vz[0:1, :], scalar2=None,
                        op0=mybir.AluOpType.add)

        ft = {}

        def load_ft(h):
            ft[h] = ftp.tile([128, CB, N // 2], F8, tag="ft", name="ft")
            ftv = featT_full[h].rearrange(
                "(co cb p) n -> cb p co n", co=NCORES, p=128)
            for cbi in range(CB):
                nc.sync.dma_start(
                    out=ft[h][:, cbi, :].rearrange(
                        "p (co nl) -> p co nl", co=NCORES),
                    in_=ftv[cbi])

        def qkv_half(h):
            base = h * (N // 2)
            for jj in range(2):
                sl = slice(base + jj * 512, base + (jj + 1) * 512)
                fsl = slice(jj * 512, (jj + 1) * 512)
                pq = qkvps.tile([128, 512], F32, tag="pqkv", name="pq")
                for cbi in range(CB):
                    nc.tensor.matmul(pq, wq_t[:, cbi, :], ft[h][:, cbi, fsl],
                                     start=(cbi == 0), stop=(cbi == CB - 1))
                nc.vector.tensor_scalar(
                    out=qT[:, sl], in0=pq, scalar1=qb2, scalar2=None,
                    op0=mybir.AluOpType.add)
                pk = qkvps.tile([128, 512], F32, tag="pqkv", name="pk")
                for cbi in range(CB):
                    nc.tensor.matmul(pk, wk_t[:, cbi, :], ft[h][:, cbi, fsl],
                                     start=(cbi == 0), stop=(cbi == CB - 1))
                nc.vector.tensor_scalar(
                    out=kT[:, sl], in0=pk, scalar1=kb2, scalar2=None,
                    op0=mybir.AluOpType.add)
            for lt in range(HMT):
                mt = h * HMT + lt
                fsl = slice(lt * 128, (lt + 1) * 128)
                pv = qkvps.tile([128, HD], F32, tag="pqkv", name="pv")
                for cbi in range(CB):
                    nc.tensor.matmul(pv, ft[h][:, cbi, fsl], wv_t[:, cbi, :],
                                     start=(cbi == 0), stop=False)
                nc.tensor.matmul(pv, ones_row, vb_t, start=False, stop=True)
                nc.vector.tensor_scalar(
                    out=v_t[:, mt, :], in0=pv, scalar1=vz, scalar2=None,
                    op0=mybir.AluOpType.add)

        def attn_chunk(j):
            """banded scoresT -> exp -> den -> out[q,d]/den for query chunk j."""
            sl = slice(j * 512, (j + 1) * 512)
            band = list(_band(j, dense))
            nb = len(band)
            expj = expbuf.tile([128, BW, 512], F8, tag="expj", name="expj")
            for bi_, mt in enumerate(band):
                msl = slice(mt * 128, (mt + 1) * 128)
                ps = scoreps.tile([128, 512], F32, tag="pscore", name="ps")
                nc.tensor.matmul(ps, kT[:, msl], qT[:, sl],
                                 start=True, stop=False)
                nc.tensor.matmul(ps, oha_t[:, msl], ohb_t[:, sl],
                                 start=False, stop=True)
                nc.scalar.activation(
                    out=expj[:, bi_, :], in_=ps,
                    func=mybir.ActivationFunctionType.Exp, bias=vz)
            pden = denps.tile([1, 512], F32, tag="pden", name="pden")
            for bi_ in range(nb):
                nc.tensor.matmul(pden, ones_col, expj[:, bi_, :],
                                 start=(bi_ == 0), stop=(bi_ == nb - 1))
            nc.vector.tensor_scalar(
                out=den_sb[:, sl], in0=pden, scalar1=vz[0:1, :], scalar2=None,
                op0=mybir.AluOpType.add)
            for qt in range(4):
                t = j * 4 + qt
                pdt = denps.tile([128, 1], F32, tag="pdt", name="pdt")
                nc.tensor.matmul(pdt, den_sb[0:1, t * 128:(t + 1) * 128],
                                 one_1x1, start=True, stop=True)
                nc.vector.tensor_scalar(
                    out=denT[:, t:t + 1], in0=pdt, scalar1=vz, scalar2=None,
                    op0=mybir.AluOpType.add)
            nc.vector.reciprocal(out=recipT[:, j * 4:j * 4 + 4],
                                 in_=denT[:, j * 4:j * 4 + 4])
            for qt in range(4):
                t = j * 4 + qt
                qsl = slice(qt * 128, (qt + 1) * 128)
                pout = avdps.tile([128, HD], F32, tag="pout", name="pout")
                for bi_, mt in enumerate(band):
                    nc.tensor.matmul(pout, expj[:, bi_, qsl], v_t[:, mt, :],
                                     start=(bi_ == 0), stop=(bi_ == nb - 1))
                nc.vector.tensor_scalar(
                    out=outb[:, t, :], in0=pout,
                    scalar1=recipT[:, t:t + 1], scalar2=None,
                    op0=mybir.AluOpType.mult)

        def route_half(k):
            """AllToAll outb tiles [8k, 8k+8) to their owners (tile k)."""
            a2av = a2a_in[k].rearrange("(r p) d -> p r d", r=NCORES, p=128)
            nc.gpsimd.dma_start(
                out=a2av, in_=outb[:, k * NCORES:(k + 1) * NCORES, :])
            nc.gpsimd.collective_compute(
                "AllToAll", mybir.AluOpType.bypass, replica_groups=groups,
                ins=[a2a_in[k][:]], outs=[a2a_out[k][:]])
            nc.gpsimd.dma_start(
                out=out_all[:, k, :].rearrange("p (h d) -> p h d", h=NH),
                in_=a2a_out[k].rearrange("(h p) d -> p h d", p=128))

        def delta_k(k):
            for h in range(NH):
                pt = scoreps.tile([128, 128], BF16, tag="pscore",
                                  name="trps2")
                nc.tensor.transpose(
                    pt, out_all[:, k, h * 128:(h + 1) * 128], identb)
                nc.vector.tensor_copy(out=out_allT[:, k, h, :], in_=pt)
            for cs in range(2):
                csl = slice(cs * 512, (cs + 1) * 512)
                pdl = qkvps.tile([128, 512], F32, tag="pqkv", name="pdl")
                for h in range(NH):
                    nc.tensor.matmul(pdl, out_allT[:, k, h, :],
                                     wo_t[:, h, csl],
                                     start=(h == 0), stop=(h == NH - 1))
                db = p3.tile([128, 512], F32, tag="db", name="db")
                nc.vector.tensor_add(out=db, in0=pdl, in1=bo_t[:, csl])
                nc.vector.tensor_scalar(
                    out=gd[:, k, csl], in0=db,
                    scalar1=g_t[:, k:k + 1], scalar2=None,
                    op0=mybir.AluOpType.mult)

        def adds(k, pre):
            for wc in range(WC):
                i = k * WC + wc
                if i < len(pre):
                    xt = pre[i]
                else:
                    xt = xin.tile([128, WC, C], BF16, tag="xt", name="xt")
                    xq(i).dma_start(
                        out=xt,
                        in_=xs[k * 128:(k + 1) * 128,
                               wc * WC:(wc + 1) * WC, :])
                gslice = gd[:, k, :]
                nc.vector.tensor_tensor(
                    out=xt, in0=xt,
                    in1=gslice[:, None, :].to_broadcast((128, WC, C)),
                    op=mybir.AluOpType.add)
                (nc.sync if i % 2 == 0 else nc.scalar).dma_start(
                    out=out[k * 128:(k + 1) * 128, wc * WC:(wc + 1) * WC, :],
                    in_=xt)

        # ---------------- pipeline emission ----------------
        pool_loads_reduces(0)
        ln_transpose_ag(0)          # AG(A)
        pool_loads_reduces(1)
        ln_transpose_ag(1)          # AG(B) triggers right after half-B LN
        load_ft(0)
        qkv_half(0)
        if not dense:
            attn_chunk(0)           # band of chunk 0 lives in half A

        # release phase-1 scratch (x tiles keep riding the xin ring)
        p1one_cm.__exit__(None, None, None)
        p1_cm.__exit__(None, None, None)

        load_ft(1)
        # gate the add-phase prefetch behind AG(B): the AllGather is on the
        # critical path and must not share HBM with the x prefetch stream.
        pin2 = attper.tile([128, 1], F32, name="pin2")
        nc.vector.scalar_tensor_tensor(
            out=pin2, in0=ft[1][:, 0, 0:1], scalar=0.0, in1=ft[1][:, 0, 0:1],
            op0=mybir.AluOpType.mult, op1=mybir.AluOpType.mult)
        pre = []
        for i in range(pf):
            k, wc = divmod(i, WC)
            xt = xin.tile([128, WC, C], BF16, tag="xt", name="xt")
            nc.vector.tensor_copy(out=xt[0:1, 0, 0:1], in_=pin2[0:1, :])
            xq2(i).dma_start(
                out=xt,
                in_=xs[k * 128:(k + 1) * 128, wc * WC:(wc + 1) * WC, :])
            pre.append(xt)
        qkv_half(1)
        if dense:
            attn_chunk(0)
        attn_chunk(1)
        route_half(0)               # A2A(tile 0) while chunks 2-3 compute
        attn_chunk(2)
        attn_chunk(3)
        route_half(1)               # A2A(tile 1) while delta0/adds0 run
        delta_k(0)
        adds(0, pre)
        delta_k(1)
        adds(1, pre)
        xin_cm.__exit__(None, None, None)

    _install_wait_split(nc)
    return nc


_NC_CACHE = {}


def get_program(dense=False):
    if dense not in _NC_CACHE:
        _NC_CACHE[dense] = build_program(dense)
    return _NC_CACHE[dense]


def _band_ok(bi_sorted):
    """Check the static band [4j-1, 4j+4] covers every group of each chunk."""
    counts = np.bincount(bi_sorted, minlength=NG)
    if counts.max() > GCAP:
        return False
    s = 0
    for g in range(NG):
        e = s + counts[g]
        if counts[g]:
            for j in range(NQC):
                if s < (j + 1) * 512 and e > j * 512:  # intersects chunk j
                    lo, hi = max(0, 4 * j - 1) * 128, min(MT, 4 * j + 5) * 128
                    if s < lo or e > hi:
                        return False
        s = e
    return True


def _shard_rows(c):
    """Sorted-row indices owned by core c: global tiles c and c+8."""
    return np.r_[128 * c:128 * (c + 1), 1024 + 128 * c:1024 + 128 * (c + 1)]


def prepare_in_maps(x, batch_indices, ln_w, ln_b, in_proj_w, in_proj_b,
                    out_proj_w, out_proj_b, gamma):
    x = np.asarray(x, dtype=np.float32)
    bi_orig = np.asarray(batch_indices).astype(np.int64)
    perm = np.argsort(bi_orig, kind="stable")
    bi = bi_orig[perm]
    dense = not _band_ok(bi)
    ln_w = np.asarray(ln_w, np.float32)
    ln_b = np.asarray(ln_b, np.float32)
    ipw = np.asarray(in_proj_w, np.float32) * ln_w[None, :]
    ipb = (np.asarray(in_proj_b, np.float32)
           + np.asarray(in_proj_w, np.float32) @ ln_b)
    opw = np.asarray(out_proj_w, np.float32)
    opb = np.ascontiguousarray(np.asarray(out_proj_b, np.float32))
    gamma = np.asarray(gamma, np.float32)
    f8 = ml_dtypes.float8_e4m3

    oh = (bi[None, :] == np.arange(NG, dtype=np.int64)[:, None]).astype(np.float32)
    oha = np.ascontiguousarray((MASK_NEG * oh).astype(f8))
    ohb = np.ascontiguousarray((1.0 - oh).astype(f8))
    counts = np.bincount(bi, minlength=NG)
    g = np.where(counts[bi] > 1, gamma[0], np.float32(0.0)).astype(np.float32)

    wot = np.ascontiguousarray(opw.T.astype(f8))
    # [N, C, HW] -> sorted -> [N, HW, C] bf16
    xr = np.ascontiguousarray(
        x.reshape(N, C, HW)[perm].transpose(0, 2, 1)).astype(ml_dtypes.bfloat16)
    in_maps = []
    for c in range(NCORES):
        h0 = c * HD
        rows = _shard_rows(c)
        in_maps.append({
            "xs": np.ascontiguousarray(xr[rows]),
            "wq": np.ascontiguousarray((ipw[h0:h0 + HD, :].T
                   / np.sqrt(np.float32(HD))).astype(f8)),
            "wk": np.ascontiguousarray(ipw[C + h0:C + h0 + HD, :].T.astype(f8)),
            "wv": np.ascontiguousarray(ipw[2 * C + h0:2 * C + h0 + HD, :].T.astype(f8)),
            "qb": np.ascontiguousarray((ipb[h0:h0 + HD] / np.sqrt(np.float32(HD))).reshape(HD, 1)),
            "kb": np.ascontiguousarray(ipb[C + h0:C + h0 + HD].reshape(HD, 1)),
            "vb": np.ascontiguousarray(ipb[2 * C + h0:2 * C + h0 + HD].reshape(1, HD).astype(ml_dtypes.bfloat16)),
            "wot": wot,
            "oha": oha,
            "ohb": ohb,
            "bo": opb,
            "gsh": np.ascontiguousarray(g[rows]),
        })
    return in_maps, perm, dense


def assemble(results, perm):
    y_sorted = np.empty((N, C, 8, 8), np.float32)
    for c in range(NCORES):
        # [NS, HW, C] bf16 -> [NS, C, HW] f32
        y_sorted[_shard_rows(c)] = (
            results[c]["out"].astype(np.float32)
            .transpose(0, 2, 1).reshape(NS, C, 8, 8))
    y = np.empty_like(y_sorted)
    y[perm] = y_sorted
    return y


def kernel(**inputs) -> np.ndarray:
    in_maps, perm, dense = prepare_in_maps(**inputs)
    nc = get_program(dense)
    res = run_bass_kernel_spmd(nc, in_maps, list(range(NCORES)), trace=False)
    return assemble(res.results, perm)


# revision 24
# speedup vs baseline: 1.0861x; 1.0861x over previous
"""Trainium2 Bass kernel for nn_BlockCorrelation (sparse/block attention).

Self-contained: accepts FULL inputs, shards across 8 NeuronCores internally,
returns the FULL output.

Math (see reference):
    feat = x.mean((2,3)); feat_n = LN(feat)*ln_w + ln_b
    qkv  = feat_n @ in_proj_w.T + in_proj_b  -> 8-head attention with a
    block-diagonal mask (attend only within equal batch_indices groups)
    out  = attn_out @ out_proj_w.T + out_proj_b
    y    = x + where(group_count>1, gamma*out, 0)[..., None, None]

Distribution / key optimizations:
  - rows sorted by group on host; core c owns sorted tiles c and c+8
    (interleaved, so each half of the sorted rows is one AllGather).
  - x is shipped bf16 and transposed to [rows, HW, C]: with C innermost both
    the pooling tree (packed pairwise adds over HW) and the y = x + g*delta
    broadcast-add run in the DVE 2x mode; a [128,8,1024] chunk is one 16 KB
    contiguous run per partition.  y returns bf16 (host upcasts/permutes).
  - x rides the scalar-engine DMA queue alone (weights/masks on the gpsimd
    queue) so the stream saturates from t=0; the add-phase prefetch reuses
    the same SBUF ring and is gated behind AG(B) so the critical-path
    AllGather never shares HBM bandwidth with the prefetch stream.
  - a tiny warmup AllGather at t=0 bootstraps the CC channel.
  - attention is head-parallel (core h = head h over all rows), entirely in
    fp8 (e4m3) on the PE: LN'd featT is AllGathered in fp8 (1 MiB), QKV /
    banded scores + one-hot mask / exp / ones-matmul denominator all fp8;
    out[q,d] = expT @ v is divided by the denominator on the head core and
    routed to row owners by two AllToAlls (256 KB bf16 each, one per row
    tile) so out_proj/gamma/adds for tile 0 start while tile 1's attention
    is still finishing.  Attention feeds only gamma*delta (~0.5% of |y|),
    so fp8 error is negligible.
  - 1-element weight "touches" and zero-bias pins force the static Tile
    scheduler to keep the half-B LN/AllGather chain ahead of the
    AllGather-gated attention ops in each in-order engine stream.
"""

import json
import sys

if "/opt/trn_rl_repo" not in sys.path:
    sys.path.insert(0, "/opt/trn_rl_repo")

import ml_dtypes
import numpy as np

import concourse.bass as bass
import concourse.mybir as mybir
import concourse.tile as tile
from concourse.bass_utils import run_bass_kernel_spmd
from concourse.masks import make_identity

F32 = mybir.dt.float32
BF16 = mybir.dt.bfloat16
F8 = mybir.dt.float8e4

# Problem shape (hardcoded per contract)
N, C, HW = 2048, 1024, 64
NH, HD = 8, 128
NG = 32
EPS = 1e-5
NCORES = 8
NS = N // NCORES          # 256 rows per core
NT = 2                    # own row tiles of 128 (tile k = global tile c+8k)
CB = C // 128             # 8 channel blocks
MT = N // 128             # 16 global key tiles
HMT = MT // 2             # 8 key tiles per half
NQC = N // 512            # 4 query chunks of 512
MASK_NEG = -50.0          # additive mask magnitude
GCAP = 128                # band attention assumes group fits in one m-tile
WC = 8                    # hw-chunks per row tile (64 hw / 8)
PF = 7                    # add-phase x prefetch depth (chunks of 2 MiB)


def _band(j, dense):
    if dense:
        return range(MT)
    return range(max(0, 4 * j - 1), min(MT, 4 * j + 5))


# ---------------------------------------------------------------------------
# walrus workaround: this build rejects >1 sem wait per instruction in some
# CTRL lowerings; split excess on_wait entries onto preceding same-engine
# EventSemaphore instructions (the exact shape wait_ge() lowers to).
def _split_waits_json(j, max_waits=1):
    for f in j.get("functions", []):
        for bb in f.get("blocks", []):
            out = []
            for ins in bb.get("instructions", []):
                si = ins.get("sync_info")
                waits = (si or {}).get("on_wait") or []
                if len(waits) > max_waits:
                    head, tail = waits[:-max_waits], waits[-max_waits:]
                    for k, w in enumerate(head):
                        out.append({
                            "name": f"{ins['name']}-wsplit{k}",
                            "opcode": "EventSemaphore",
                            "engine": ins["engine"],
                            "ins": [],
                            "outs": [],
                            "debug": ins.get("debug", 0),
                            "sync_info": {"on_update": [], "on_wait": [w]},
                        })
                    si["on_wait"] = tail
                out.append(ins)
            bb["instructions"] = out
    return j


def _install_wait_split(nc, max_waits=1):
    def to_json_bytes_fixed():
        j = json.loads(mybir.module_to_json_bytes(nc.m))
        return json.dumps(_split_waits_json(j, max_waits)).encode()

    nc.to_json_bytes = to_json_bytes_fixed


def _bcast_ap(ap, parts=128):
    """DRAM AP broadcast across partitions (stride-0 partition dim)."""
    return bass.AP(tensor=ap.tensor, offset=ap.offset, ap=[[0, parts]] + ap.ap)


# ---------------------------------------------------------------------------
def build_program(dense=False):
    nc = bass.Bass(num_devices=NCORES)

    # --- per-core parameters (SPMD: same program, different data) ---
    xs = nc.declare_dram_parameter("xs", [NS, HW, C], BF16, isOutput=False)
    wq = nc.declare_dram_parameter("wq", [C, HD], F8, isOutput=False)
    wk = nc.declare_dram_parameter("wk", [C, HD], F8, isOutput=False)
    wv = nc.declare_dram_parameter("wv", [C, HD], F8, isOutput=False)
    qb = nc.declare_dram_parameter("qb", [HD, 1], F32, isOutput=False)  # pre-scaled by 1/sqrt(HD)
    kb = nc.declare_dram_parameter("kb", [HD, 1], F32, isOutput=False)
    vb = nc.declare_dram_parameter("vb", [1, HD], BF16, isOutput=False)
    wot = nc.declare_dram_parameter("wot", [C, C], F8, isOutput=False)  # out_proj_w.T (ci, co)
    oha = nc.declare_dram_parameter("oha", [NG, N], F8, isOutput=False)
    ohb = nc.declare_dram_parameter("ohb", [NG, N], F8, isOutput=False)
    bo = nc.declare_dram_parameter("bo", [C], F32, isOutput=False)
    gsh = nc.declare_dram_parameter("gsh", [NS], F32, isOutput=False)
    out = nc.declare_dram_parameter("out", [NS, HW, C], BF16, isOutput=True)

    # --- internal DRAM for collectives ---
    warm_in = nc.dram_tensor("warm_in", [16], BF16)
    warm_out = nc.dram_tensor("warm_out", [NCORES * 16], BF16,
                              addr_space="Shared")
    featT_sh = [nc.dram_tensor(f"featT_sh{h}", [C, 128], F8)
                for h in range(NT)]
    featT_full = [
        nc.dram_tensor(f"featT_full{h}", [NCORES * C, 128], F8,
                       addr_space="Shared")
        for h in range(NT)
    ]
    a2a_in = [nc.dram_tensor(f"a2a_in{k}", [N // 2, HD], BF16)
              for k in range(NT)]
    a2a_out = [nc.dram_tensor(f"a2a_out{k}", [N // 2, HD], BF16)
               for k in range(NT)]

    groups = [list(range(NCORES))]
    inv_sqrt_hd = 1.0 / float(np.sqrt(np.float32(HD)))
    BW = MT if dense else 6
    pf = 4 if dense else PF

    with tile.TileContext(nc, num_cores=NCORES) as tc:
      with (
        tc.tile_pool(name="singles", bufs=1) as singles,
        tc.tile_pool(name="attper", bufs=1) as attper,
        tc.tile_pool(name="qkvw", bufs=1) as qkvw,
        tc.tile_pool(name="maskp", bufs=1) as maskp,
        tc.tile_pool(name="expbuf", bufs=2) as expbuf,
        tc.tile_pool(name="p3", bufs=1) as p3,
        tc.tile_pool(name="ftp", bufs=2) as ftp,
        tc.tile_pool(name="qkvps", bufs=2, space="PSUM") as qkvps,
        tc.tile_pool(name="scoreps", bufs=2, space="PSUM") as scoreps,
        tc.tile_pool(name="avdps", bufs=2, space="PSUM") as avdps,
        tc.tile_pool(name="denps", bufs=1, space="PSUM") as denps,
      ):
        # ---- CC-channel warmup: bootstrap the collectives rendezvous now so
        # the first real AllGather doesn't pay it.
        nc.gpsimd.collective_compute(
            "AllGather", mybir.AluOpType.bypass, replica_groups=groups,
            ins=[warm_in[:]], outs=[warm_out[:]])

        # ---- constants / weights (weights on the gpsimd DMA queue; the
        # scalar+tensor queues carry only the x stream) ----
        ident = singles.tile([128, 128], F32)
        make_identity(nc, ident)
        identb = singles.tile([128, 128], BF16)
        make_identity(nc, identb)
        ones_col = singles.tile([128, 1], F8)
        nc.vector.memset(ones_col, 1.0)
        one_1x1 = singles.tile([1, 1], F32)
        nc.vector.memset(one_1x1, 1.0)
        ones_row = singles.tile([1, 128], BF16)
        nc.vector.memset(ones_row, 1.0)

        wq_t = qkvw.tile([128, CB, HD], F8)
        nc.gpsimd.dma_start(out=wq_t, in_=wq.rearrange("(cb p) d -> p cb d", p=128))
        wk_t = qkvw.tile([128, CB, HD], F8)
        nc.gpsimd.dma_start(out=wk_t, in_=wk.rearrange("(cb p) d -> p cb d", p=128))
        wv_t = qkvw.tile([128, CB, HD], F8)
        nc.gpsimd.dma_start(out=wv_t, in_=wv.rearrange("(cb p) d -> p cb d", p=128))
        qb_t = qkvw.tile([128, 1], F32)
        nc.gpsimd.dma_start(out=qb_t, in_=qb[:])
        kb_t = qkvw.tile([128, 1], F32)
        nc.gpsimd.dma_start(out=kb_t, in_=kb[:])
        vb_t = qkvw.tile([1, 128], BF16)
        nc.gpsimd.dma_start(out=vb_t, in_=vb[:])
        wo_t = qkvw.tile([128, NH, C], F8)   # (ci within head-block, h, co)
        nc.gpsimd.dma_start(out=wo_t, in_=wot.rearrange("(h p) co -> p h co", p=128))
        oha_t = maskp.tile([128, N], F8)
        nc.vector.memset(oha_t, 0.0)
        nc.gpsimd.dma_start(out=oha_t[:NG, :], in_=oha[:])
        ohb_t = maskp.tile([128, N], F8)
        nc.vector.memset(ohb_t, 0.0)
        nc.gpsimd.dma_start(out=ohb_t[:NG, :], in_=ohb[:])
        bo_t = p3.tile([128, C], F32)
        nc.gpsimd.dma_start(out=bo_t, in_=_bcast_ap(bo[:]))
        g_t = p3.tile([128, NT], F32)
        nc.gpsimd.dma_start(out=g_t, in_=gsh.rearrange("(t p) -> p t", p=128))

        # ---- persistent attention state ----
        qT = attper.tile([128, N], F8, name="qT")
        kT = attper.tile([128, N], F8, name="kT")
        v_t = attper.tile([128, MT, HD], F8, name="v")
        outb = attper.tile([128, MT, HD], BF16, name="outb")
        denT = attper.tile([128, MT], F32, name="denT")
        recipT = attper.tile([128, MT], F32, name="recipT")
        den_sb = attper.tile([1, N], F32, name="den_sb")
        out_all = attper.tile([128, NT, C], BF16, name="out_all")
        out_allT = attper.tile([128, NT, NH, 128], F8, name="out_allT")
        gd = p3.tile([128, NT, C], BF16, name="gd")
        # scheduler pins: qb2/kb2/vz depend on half-B's LN output, so the
        # static per-engine schedule cannot hoist attention-phase DVE/Act
        # ops above the half-B pooling/LN stream (a hoisted wait stalls the
        # in-order engine and delays AG(B) by ~30us).
        qb2 = attper.tile([128, 1], F32, name="qb2")
        kb2 = attper.tile([128, 1], F32, name="kb2")
        vz = attper.tile([128, 1], F32, name="vz")

        # ---- phase-1 pools (exit before the prefetch pool opens) ----
        xin_cm = tc.tile_pool(name="xin", bufs=pf)
        p1_cm = tc.tile_pool(name="p1", bufs=2)
        p1one_cm = tc.tile_pool(name="p1one", bufs=1)
        xin = xin_cm.__enter__()
        p1 = p1_cm.__enter__()
        p1one = p1one_cm.__enter__()

        eps_t = p1one.tile([128, 1], F32)
        nc.vector.memset(eps_t, EPS * HW * HW)  # LN on sums: eps * 64^2

        faccs = {}

        def xq(i):
            return nc.scalar

        def xq2(i):
            return nc.sync

        def pool_loads_reduces(k):
            """Stream own tile k as [128, 8hw, 1024c] chunks; 2x-packed
            pairwise-add tree over hw; accumulate channel sums in bf16."""
            facc = p1.tile([128, C], BF16, tag="facc", name="facc")
            faccs[k] = facc
            def finish(xt, wc):
                nc.vector.tensor_tensor(
                    out=xt[:, 0:2, :], in0=xt[:, 0:2, :],
                    in1=xt[:, 2:4, :], op=mybir.AluOpType.add)
                if wc == 0:
                    nc.vector.tensor_tensor(
                        out=facc, in0=xt[:, 0, :], in1=xt[:, 1, :],
                        op=mybir.AluOpType.add)
                else:
                    t3 = p1.tile([128, C], BF16, tag="t3", name="t3")
                    nc.vector.tensor_tensor(
                        out=t3, in0=xt[:, 0, :], in1=xt[:, 1, :],
                        op=mybir.AluOpType.add)
                    nc.vector.tensor_tensor(
                        out=facc, in0=facc, in1=t3,
                        op=mybir.AluOpType.add)

            prev = None
            with nc.allow_low_precision(reason="pool partials in bf16"):
                for wc in range(WC):
                    xt = xin.tile([128, WC, C], BF16, tag="xt", name="xt")
                    xq(wc).dma_start(
                        out=xt,
                        in_=xs[k * 128:(k + 1) * 128,
                               wc * WC:(wc + 1) * WC, :])
                    nc.vector.tensor_tensor(
                        out=xt[:, 0:4, :], in0=xt[:, 0:4, :],
                        in1=xt[:, 4:8, :], op=mybir.AluOpType.add)
                    if prev is not None:
                        finish(*prev)
                    prev = (xt, wc)
                finish(*prev)

        def ln_transpose_ag(k):
            """LN over channels (on sums), transpose, AG half k (fp8)."""
            facc = faccs[k]
            stats = p1.tile([128, 2, 6], F32, tag="stats", name="stats")
            for sg in range(2):
                nc.vector.bn_stats(out=stats[:, sg, :],
                                   in_=facc[:, sg * 512:(sg + 1) * 512])
            mv = p1.tile([128, 2], F32, tag="mv", name="mv")
            nc.vector.bn_aggr(out=mv, in_=stats)
            std = p1.tile([128, 1], F32, tag="std", name="std")
            nc.scalar.activation(
                out=std, in_=mv[:, 1:2],
                func=mybir.ActivationFunctionType.Sqrt, bias=eps_t, scale=1.0)
            rstd = p1.tile([128, 1], F32, tag="rstd", name="rstd")
            nc.vector.reciprocal(out=rstd, in_=std)
            featn = p1.tile([128, C], F32, tag="featn", name="featn")
            nc.vector.tensor_scalar(
                out=featn, in0=facc, scalar1=mv[:, 0:1], scalar2=rstd,
                op0=mybir.AluOpType.subtract, op1=mybir.AluOpType.mult)
            fTb = p1.tile([128, CB, 128], F8, tag="fTb", name="fTb")
            for cbi in range(CB):
                pt = scoreps.tile([128, 128], F32, tag="pscore", name="trps")
                nc.tensor.transpose(pt, featn[:, cbi * 128:(cbi + 1) * 128], ident)
                nc.vector.tensor_copy(out=fTb[:, cbi, :], in_=pt)
            nc.gpsimd.dma_start(
                out=featT_sh[k].rearrange("(cb p) n -> p cb n", p=128),
                in_=fTb)
            nc.gpsimd.collective_compute(
                "AllGather", mybir.AluOpType.bypass, replica_groups=groups,
                ins=[featT_sh[k][:]], outs=[featT_full[k][:]])
            if k == NT - 1:
                nc.vector.scalar_tensor_tensor(
                    out=vz, in0=fTb[:, 0, 0:1], scalar=0.0, in1=fTb[:, 0, 0:1],
                    op0=mybir.AluOpType.mult, op1=mybir.AluOpType.mult)
                nc.vector.scalar_tensor_tensor(
                    out=qb2, in0=fTb[:, 0, 0:1], scalar=0.0, in1=qb_t,
                    op0=mybir.AluOpType.mult, op1=mybir.AluOpType.add)
                nc.vector.scalar_tensor_tensor(
                    out=kb2, in0=fTb[:, 0, 0:1], scalar=0.0, in1=kb_t,
                    op0=mybir.AluOpType.mult, op1=mybir.AluOpType.add)
                # 1-element touches: every attention matmul reads one of
                # these weights, so the tensor stream cannot be scheduled
                # ahead of half-B's LN/transposes (which gate AG(B)).
                for wt in (wq_t, wk_t, wv_t):
                    nc.vector.tensor_scalar(
                        out=wt[0:1, 0, 0:1], in0=wt[0:1, 0, 0:1],
                        scalar1=vz[0:1, :], scalar2=None,
                        op0=mybir.AluOpType.add)

        ft = {}

        def load_ft(h):
            ft[h] = ftp.tile([128, CB, N // 2], F8, tag="ft", name="ft")
            ftv = featT_full[h].rearrange(
                "(co cb p) n -> cb p co n", co=NCORES, p=128)
            for cbi in range(CB):
                nc.sync.dma_start(
                    out=ft[h][:, cbi, :].rearrange(
                        "p (co nl) -> p co nl", co=NCORES),
                    in_=ftv[cbi])

        def qkv_half(h):
            base = h * (N // 2)
            for jj in range(2):
                sl = slice(base + jj * 512, base + (jj + 1) * 512)
                fsl = slice(jj * 512, (jj + 1) * 512)
                pq = qkvps.tile([128, 512], F32, tag="pqkv", name="pq")
                for cbi in range(CB):
                    nc.tensor.matmul(pq, wq_t[:, cbi, :], ft[h][:, cbi, fsl],
                                     start=(cbi == 0), stop=(cbi == CB - 1))
                nc.vector.tensor_scalar(
                    out=qT[:, sl], in0=pq, scalar1=qb2, scalar2=None,
                    op0=mybir.AluOpType.add)
                pk = qkvps.tile([128, 512], F32, tag="pqkv", name="pk")
                for cbi in range(CB):
                    nc.tensor.matmul(pk, wk_t[:, cbi, :], ft[h][:, cbi, fsl],
                                     start=(cbi == 0), stop=(cbi == CB - 1))
                nc.vector.tensor_scalar(
                    out=kT[:, sl], in0=pk, scalar1=kb2, scalar2=None,
                    op0=mybir.AluOpType.add)
            for lt in range(HMT):
                mt = h * HMT + lt
                fsl = slice(lt * 128, (lt + 1) * 128)
                pv = qkvps.tile([128, HD], F32, tag="pqkv", name="pv")
                for cbi in range(CB):
                    nc.tensor.matmul(pv, ft[h][:, cbi, fsl], wv_t[:, cbi, :],
                                     start=(cbi == 0), stop=False)
                nc.tensor.matmul(pv, ones_row, vb_t, start=False, stop=True)
                nc.vector.tensor_scalar(
                    out=v_t[:, mt, :], in0=pv, scalar1=vz, scalar2=None,
                    op0=mybir.AluOpType.add)

        def attn_chunk(j):
            """banded scoresT -> exp -> den -> out[q,d]/den for query chunk j."""
            sl = slice(j * 512, (j + 1) * 512)
            band = list(_band(j, dense))
            nb = len(band)
            expj = expbuf.tile([128, BW, 512], F8, tag="expj", name="expj")
            for bi_, mt in enumerate(band):
                msl = slice(mt * 128, (mt + 1) * 128)
                ps = scoreps.tile([128, 512], F32, tag="pscore", name="ps")
                nc.tensor.matmul(ps, kT[:, msl], qT[:, sl],
                                 start=True, stop=False)
                nc.tensor.matmul(ps, oha_t[:, msl], ohb_t[:, sl],
                                 start=False, stop=True)
                nc.scalar.activation(
                    out=expj[:, bi_, :], in_=ps,
                    func=mybir.ActivationFunctionType.Exp, bias=vz)
            pden = denps.tile([1, 512], F32, tag="pden", name="pden")
            for bi_ in range(nb):
                nc.tensor.matmul(pden, ones_col, expj[:, bi_, :],
                                 start=(bi_ == 0), stop=(bi_ == nb - 1))
            nc.vector.tensor_scalar(
                out=den_sb[:, sl], in0=pden, scalar1=vz[0:1, :], scalar2=None,
                op0=mybir.AluOpType.add)
            for qt in range(4):
                t = j * 4 + qt
                pdt = denps.tile([128, 1], F32, tag="pdt", name="pdt")
                nc.tensor.matmul(pdt, den_sb[0:1, t * 128:(t + 1) * 128],
                                 one_1x1, start=True, stop=True)
                nc.vector.tensor_scalar(
                    out=denT[:, t:t + 1], in0=pdt, scalar1=vz, scalar2=None,
                    op0=mybir.AluOpType.add)
            nc.vector.reciprocal(out=recipT[:, j * 4:j * 4 + 4],
                                 in_=denT[:, j * 4:j * 4 + 4])
            for qt in range(4):
                t = j * 4 + qt
                qsl = slice(qt * 128, (qt + 1) * 128)
                pout = avdps.tile([128, HD], F32, tag="pout", name="pout")
                for bi_, mt in enumerate(band):
                    nc.tensor.matmul(pout, expj[:, bi_, qsl], v_t[:, mt, :],
                                     start=(bi_ == 0), stop=(bi_ == nb - 1))
                nc.vector.tensor_scalar(
                    out=outb[:, t, :], in0=pout,
                    scalar1=recipT[:, t:t + 1], scalar2=None,
                    op0=mybir.AluOpType.mult)

        def route_half(k):
            """AllToAll outb tiles [8k, 8k+8) to their owners (tile k)."""
            a2av = a2a_in[k].rearrange("(r p) d -> p r d", r=NCORES, p=128)
            nc.gpsimd.dma_start(
                out=a2av, in_=outb[:, k * NCORES:(k + 1) * NCORES, :])
            nc.gpsimd.collective_compute(
                "AllToAll", mybir.AluOpType.bypass, replica_groups=groups,
                ins=[a2a_in[k][:]], outs=[a2a_out[k][:]])
            nc.gpsimd.dma_start(
                out=out_all[:, k, :].rearrange("p (h d) -> p h d", h=NH),
                in_=a2a_out[k].rearrange("(h p) d -> p h d", p=128))

        def delta_k(k):
            for h in range(NH):
                pt = scoreps.tile([128, 128], BF16, tag="pscore",
                                  name="trps2")
                nc.tensor.transpose(
                    pt, out_all[:, k, h * 128:(h + 1) * 128], identb)
                nc.vector.tensor_copy(out=out_allT[:, k, h, :], in_=pt)
            for cs in range(2):
                csl = slice(cs * 512, (cs + 1) * 512)
                pdl = qkvps.tile([128, 512], F32, tag="pqkv", name="pdl")
                for h in range(NH):
                    nc.tensor.matmul(pdl, out_allT[:, k, h, :],
                                     wo_t[:, h, csl],
                                     start=(h == 0), stop=(h == NH - 1))
                db = p3.tile([128, 512], F32, tag="db", name="db")
                nc.vector.tensor_add(out=db, in0=pdl, in1=bo_t[:, csl])
                nc.vector.tensor_scalar(
                    out=gd[:, k, csl], in0=db,
                    scalar1=g_t[:, k:k + 1], scalar2=None,
                    op0=mybir.AluOpType.mult)

        def adds(k, pre):
            for wc in range(WC):
                i = k * WC + wc
                if i < len(pre):
                    xt = pre[i]
                else:
                    xt = xin.tile([128, WC, C], BF16, tag="xt", name="xt")
                    xq(i).dma_start(
                        out=xt,
                        in_=xs[k * 128:(k + 1) * 128,
                               wc * WC:(wc + 1) * WC, :])
                gslice = gd[:, k, :]
                nc.vector.tensor_tensor(
                    out=xt, in0=xt,
                    in1=gslice[:, None, :].to_broadcast((128, WC, C)),
                    op=mybir.AluOpType.add)
                nc.sync.dma_start(
                    out=out[k * 128:(k + 1) * 128, wc * WC:(wc + 1) * WC, :],
                    in_=xt)

        # ---------------- pipeline emission ----------------
        pool_loads_reduces(0)
        ln_transpose_ag(0)          # AG(A)
        pool_loads_reduces(1)
        ln_transpose_ag(1)          # AG(B) triggers right after half-B LN
        load_ft(0)
        qkv_half(0)
        if not dense:
            attn_chunk(0)           # band of chunk 0 lives in half A

        # release phase-1 scratch (x tiles keep riding the xin ring)
        p1one_cm.__exit__(None, None, None)
        p1_cm.__exit__(None, None, None)

        load_ft(1)
        # gate the add-phase prefetch behind AG(B): the AllGather is on the
        # critical path and must not share HBM with the x prefetch stream.
        pin2 = attper.tile([128, 1], F32, name="pin2")
        nc.vector.scalar_tensor_tensor(
            out=pin2, in0=ft[1][:, 0, 0:1], scalar=0.0, in1=ft[1][:, 0, 0:1],
            op0=mybir.AluOpType.mult, op1=mybir.AluOpType.mult)
        pre = []
        for i in range(pf):
            k, wc = divmod(i, WC)
            xt = xin.tile([128, WC, C], BF16, tag="xt", name="xt")
            nc.vector.tensor_copy(out=xt[0:1, 0, 0:1], in_=pin2[0:1, :])
            xq2(i).dma_start(
                out=xt,
                in_=xs[k * 128:(k + 1) * 128, wc * WC:(wc + 1) * WC, :])
            pre.append(xt)
        qkv_half(1)
        if dense:
            attn_chunk(0)
        attn_chunk(1)
        route_half(0)               # A2A(tile 0) while chunks 2-3 compute
        attn_chunk(2)
        attn_chunk(3)
        route_half(1)               # A2A(tile 1) while delta0/adds0 run
        delta_k(0)
        adds(0, pre)
        delta_k(1)
        adds(1, pre)
        xin_cm.__exit__(None, None, None)

    _install_wait_split(nc)
    return nc


_NC_CACHE = {}


def get_program(dense=False):
    if dense not in _NC_CACHE:
        _NC_CACHE[dense] = build_program(dense)
    return _NC_CACHE[dense]


def _band_ok(bi_sorted):
    """Check the static band [4j-1, 4j+4] covers every group of each chunk."""
    counts = np.bincount(bi_sorted, minlength=NG)
    if counts.max() > GCAP:
        return False
    s = 0
    for g in range(NG):
        e = s + counts[g]
        if counts[g]:
            for j in range(NQC):
                if s < (j + 1) * 512 and e > j * 512:  # intersects chunk j
                    lo, hi = max(0, 4 * j - 1) * 128, min(MT, 4 * j + 5) * 128
                    if s < lo or e > hi:
                        return False
        s = e
    return True


def _shard_rows(c):
    """Sorted-row indices owned by core c: global tiles c and c+8."""
    return np.r_[128 * c:128 * (c + 1), 1024 + 128 * c:1024 + 128 * (c + 1)]


def prepare_in_maps(x, batch_indices, ln_w, ln_b, in_proj_w, in_proj_b,
                    out_proj_w, out_proj_b, gamma):
    x = np.asarray(x, dtype=np.float32)
    bi_orig = np.asarray(batch_indices).astype(np.int64)
    perm = np.argsort(bi_orig, kind="stable")
    bi = bi_orig[perm]
    dense = not _band_ok(bi)
    ln_w = np.asarray(ln_w, np.float32)
    ln_b = np.asarray(ln_b, np.float32)
    ipw = np.asarray(in_proj_w, np.float32) * ln_w[None, :]
    ipb = (np.asarray(in_proj_b, np.float32)
           + np.asarray(in_proj_w, np.float32) @ ln_b)
    opw = np.asarray(out_proj_w, np.float32)
    opb = np.ascontiguousarray(np.asarray(out_proj_b, np.float32))
    gamma = np.asarray(gamma, np.float32)
    f8 = ml_dtypes.float8_e4m3

    oh = (bi[None, :] == np.arange(NG, dtype=np.int64)[:, None]).astype(np.float32)
    oha = np.ascontiguousarray((MASK_NEG * oh).astype(f8))
    ohb = np.ascontiguousarray((1.0 - oh).astype(f8))
    counts = np.bincount(bi, minlength=NG)
    g = np.where(counts[bi] > 1, gamma[0], np.float32(0.0)).astype(np.float32)

    wot = np.ascontiguousarray(opw.T.astype(f8))
    # [N, C, HW] -> sorted -> [N, HW, C] bf16
    xr = np.ascontiguousarray(
        x.reshape(N, C, HW)[perm].transpose(0, 2, 1)).astype(ml_dtypes.bfloat16)
    in_maps = []
    for c in range(NCORES):
        h0 = c * HD
        rows = _shard_rows(c)
        in_maps.append({
            "xs": np.ascontiguousarray(xr[rows]),
            "wq": np.ascontiguousarray((ipw[h0:h0 + HD, :].T
                   / np.sqrt(np.float32(HD))).astype(f8)),
            "wk": np.ascontiguousarray(ipw[C + h0:C + h0 + HD, :].T.astype(f8)),
            "wv": np.ascontiguousarray(ipw[2 * C + h0:2 * C + h0 + HD, :].T.astype(f8)),
            "qb": np.ascontiguousarray((ipb[h0:h0 + HD] / np.sqrt(np.float32(HD))).reshape(HD, 1)),
            "kb": np.ascontiguousarray(ipb[C + h0:C + h0 + HD].reshape(HD, 1)),
            "vb": np.ascontiguousarray(ipb[2 * C + h0:2 * C + h0 + HD].reshape(1, HD).astype(ml_dtypes.bfloat16)),
            "wot": wot,
            "oha": oha,
            "ohb": ohb,
            "bo": opb,
            "gsh": np.ascontiguousarray(g[rows]),
        })
    return in_maps, perm, dense


def assemble(results, perm):
    y_sorted = np.empty((N, C, 8, 8), np.float32)
    for c in range(NCORES):
        # [NS, HW, C] bf16 -> [NS, C, HW] f32
        y_sorted[_shard_rows(c)] = (
            results[c]["out"].astype(np.float32)
            .transpose(0, 2, 1).reshape(NS, C, 8, 8))
    y = np.empty_like(y_sorted)
    y[perm] = y_sorted
    return y


def kernel(**inputs) -> np.ndarray:
    in_maps, perm, dense = prepare_in_maps(**inputs)
    nc = get_program(dense)
    res = run_bass_kernel_spmd(nc, in_maps, list(range(NCORES)), trace=False)
    return assemble(res.results, perm)
